# revision 24
# baseline (speedup 1.0000x reference)
"""Trainium2 Bass kernel for nn_BaseCamera_1589137899573.

Computes PSF of a phase-mask camera:
  field = aperture * exp(i*(const_phase + spline_bias))   (4096^2, nonzero on central 2048^2)
  psf   = |IFFT2( FFT2(field) * Hs )|^2                   (Hs = ifftshift(exp(i*H_phase)))
  out   = crop 728x728, normalize by sum.

Distribution over 8 NeuronCores (bf16 compute, fp32 PSUM accumulation):
  P1: band rows (2048) split 256/core; synthesize field rows + row-FFT (radix-64
      two-stage matmul DFT, twiddles folded into per-digit stage-A matrices).
  A2A: AllToAll -> each core holds 512 spectral columns x 2048 rows.
  P2: col-FFT + H-multiply + col-IFFT per 128-column chunk. The complex H
      multiply is 2 full-width DVE muls (tables [hre;him], [him;hre]); the
      re/im recombine is absorbed into doubled inverse stage-A weights that
      accumulate in PSUM.
  A2A2 + P3: row-IFFT for 96 of the 768 band rows per core, |.|^2.
  Host: assemble, crop to 728^2, normalize.
"""

import numpy as np
from ml_dtypes import bfloat16

# ---------------- problem constants (hardcoded; must match reference) -------
N = 4096              # WAVE_RES
V = 2048              # VALID_RES (band size)
B0 = 1024             # band start (pad)
PITCH = 2e-6
SENSOR_D = N * PITCH
D1 = 0.05
D2 = 0.05
FOCAL = D1 * D2 / (D1 + D2)
WCROP = 728
LAM = 5.32e-7
UP = 2
TWO_PI = 2.0 * np.pi
K_WAVE = TWO_PI / LAM

CROP_S = N // 2 - WCROP // 2 + 1          # 1685
RHI_LO, RHI_HI = CROP_S // 64, (CROP_S + WCROP - 1) // 64   # 26, 37
NSEL = RHI_HI - RHI_LO + 1                # 12 selected high-digit values
BAND_LO = 64 * RHI_LO                     # 1664
BAND_W = 64 * NSEL                        # 768
CROP_OFF = CROP_S - BAND_LO               # 21

NC = 8                # cores
RPC = V // NC         # 256 band rows per core in P1
CPC = N // NC         # 512 spectral cols per core in P2
KCHUNK = 128          # P2 k_c chunk
NCHUNK = CPC // KCHUNK  # 4
RPC3 = BAND_W // NC   # 96 rows per core in P3

F32 = np.float32
BF16 = bfloat16


# ---------------- small host helpers ----------------------------------------
def _thomas(r):
    """diag=4 off-diag=1 tridiagonal solve, float32 to mirror reference."""
    n = r.shape[0]
    cp = np.zeros(n, np.float32)
    dp = np.zeros(n, np.float32)
    c_prev = np.float32(0.0)
    d_prev = np.float32(0.0)
    for i in range(n):
        den = np.float32(4.0) - c_prev
        c_prev = np.float32(1.0) / den
        d_prev = (r[i] - d_prev) / den
        cp[i] = c_prev
        dp[i] = d_prev
    x = np.zeros(n, np.float32)
    x_next = np.float32(0.0)
    for i in range(n - 1, -1, -1):
        x_next = dp[i] - cp[i] * x_next
        x[i] = x_next
    return x


def spline_quadrant(optim_param):
    """q[i,j] = natural-cubic-spline(mp_log) at r=sqrt((i+.5)^2+(j+.5)^2), [1024,1024]."""
    p = np.asarray(optim_param, np.float32)
    mp = np.repeat(p, UP)
    y = np.concatenate([mp, np.zeros(V // 2, np.float32)])       # len 2048
    n = y.shape[0]
    rhs = (6.0 * (y[2:].astype(np.float64) - 2.0 * y[1:-1] + y[:-2])).astype(np.float32)
    M = np.concatenate([np.zeros(1, np.float32), _thomas(rhs), np.zeros(1, np.float32)])
    half = V // 2
    coord = np.arange(half, dtype=np.float32) + 0.5
    r = np.sqrt(coord[:, None] ** 2 + coord[None, :] ** 2)
    ind = np.clip(np.floor(r).astype(np.int64), 0, n - 2)
    t = r - ind.astype(np.float32)
    y0, y1 = y[ind], y[ind + 1]
    m0, m1 = M[ind], M[ind + 1]
    b = (y1 - y0) - (2.0 * m0 + m1) / 6.0
    return y0 + t * (b + t * (m0 / 2.0 + t * (m1 - m0) / 6.0))


def bias_band(optim_param):
    """Full mirrored bias map on the 2048^2 band."""
    q = spline_quadrant(optim_param)
    row = np.concatenate([q[:, ::-1], q], axis=1)
    return np.concatenate([row[::-1, :], row], axis=0)          # [2048, 2048]


def const_phase_band():
    """(input_phase + lens_phase) mod 2pi on the 2048^2 band."""
    coords = (PITCH * (np.arange(N, dtype=np.float32) - N // 2)).astype(np.float32)
    cb = coords[B0:B0 + V].astype(np.float64)
    r2 = cb[:, None] ** 2 + cb[None, :] ** 2
    ph = np.float64(K_WAVE) * r2 * (1.0 / (2 * D1) - 1.0 / (2 * FOCAL))
    return np.mod(ph, TWO_PI).astype(np.float64)


def h_spec_planes():
    """ifftshifted transfer function exp(i*H_phase): returns (re, im) [4096,4096] f64."""
    fx = ((np.arange(1, N + 1, dtype=np.float32) - np.float32(N / 2)) / np.float32(SENSOR_D)).astype(np.float32)
    FY, FX = np.meshgrid(fx, fx, indexing="ij")
    arg = np.maximum((np.float32(1.0 / LAM)) ** 2 - FX.astype(np.float64) ** 2 - FY.astype(np.float64) ** 2, 0.0)
    w1 = np.sqrt(arg).astype(np.float32)
    hp = (np.float32(TWO_PI) * w1 * np.float32(D2)).astype(np.float32).astype(np.float64)
    hre = np.cos(hp)
    him = np.sin(hp)
    hre = np.fft.ifftshift(hre)
    him = np.fft.ifftshift(him)
    return hre, him


# ---------------- DFT stage matrices (complex->real 2x blocks) ---------------
def _c2r_lhsT(E):
    """Complex matrix E [out m, in k] -> real lhsT [2k, 2m] for out=lhsT.T@rhs.

    Input layout: partitions [re(k) | im(k)], output partitions [re(m) | im(m)].
    """
    m, k = E.shape
    W = np.zeros((2 * k, 2 * m), np.float64)
    W[:k, :m] = E.real.T
    W[k:, :m] = -E.imag.T
    W[:k, m:] = E.imag.T
    W[k:, m:] = E.real.T
    return W.astype(F32)


def stage_a_fwd_mats():
    """WA[c_lo]: [64, 128] real; contracts c_hi' (32 band-high-digits), out k_lo."""
    klo = np.arange(64)[:, None]
    chi = np.arange(32)[None, :]
    mats = []
    for c_lo in range(64):
        E = np.exp(-2j * np.pi * ((16 + chi) * klo % 64) / 64.0) \
            * np.exp(-2j * np.pi * (c_lo * klo) / 4096.0)
        mats.append(_c2r_lhsT(E))
    return np.stack(mats)                                        # [64, 64, 128]


def stage_b_fwd_mat():
    """WB: [128, 128]; contracts c_lo (64), out k_hi. Pure DFT-64."""
    khi = np.arange(64)[:, None]
    clo = np.arange(64)[None, :]
    E = np.exp(-2j * np.pi * (clo * khi % 64) / 64.0)
    return _c2r_lhsT(E)                                          # [128, 128]


def stage_a_inv_mats(scale):
    """WAI[m_lo]: [128, 128]; contracts m_hi (full 64), out r_lo, +sign, *scale."""
    rlo = np.arange(64)[:, None]
    mhi = np.arange(64)[None, :]
    mats = []
    for m_lo in range(64):
        E = np.exp(2j * np.pi * (mhi * rlo % 64) / 64.0) \
            * np.exp(2j * np.pi * (m_lo * rlo) / 4096.0) * scale
        mats.append(_c2r_lhsT(E))
    return np.stack(mats)                                        # [64, 128, 128]


def stage_a_inv_ab():
    """Doubled inverse stage-A weights absorbing the H-multiply recombine.

    mA[p] = (p<64 ? re*hre : im*him), mB[p] = (p<64 ? re*him : im*hre).
    SH_re = mA_lo - mA_hi ; SH_im = mB_lo + mB_hi.
    T_j = WAIA_j.T @ mA + WAIB_j.T @ mB  (PSUM accumulated), where
    WAIA_j = [WAI_j[0:64]; -WAI_j[0:64]], WAIB_j = [WAI_j[64:128]; WAI_j[64:128]].
    Returns [128, 64*2*128] f32 (cast later).
    """
    WAI = stage_a_inv_mats(1.0 / 4096.0)                         # [64, 128, 128]
    out = np.empty((128, 64, 2, 128), np.float32)
    for j in range(64):
        out[0:64, j, 0, :] = WAI[j][0:64]
        out[64:128, j, 0, :] = -WAI[j][0:64]
        out[0:64, j, 1, :] = WAI[j][64:128]
        out[64:128, j, 1, :] = WAI[j][64:128]
    return np.ascontiguousarray(out.reshape(128, 64 * 2 * 128))


def stage_b_inv_mat():
    """WBI: [128, 64]; contracts m_lo, out r_hi in {26..37}.
    Output partitions: re at 0:12, im at 32:44 (32-aligned for engine APs)."""
    rhi = np.arange(RHI_LO, RHI_HI + 1)[:, None]
    mlo = np.arange(64)[None, :]
    E = np.exp(2j * np.pi * (mlo * rhi % 64) / 64.0)
    W = _c2r_lhsT(E)                                             # [128, 24]
    out = np.zeros((128, 64), F32)
    out[:, 0:12] = W[:, 0:12]
    out[:, 32:44] = W[:, 12:24]
    return out


# ============================================================================
#                        host-side per-core input builders
# ============================================================================
def _syn_layout_pair(top, bot, rs):
    """[256, 2048] slabs -> [128, 8192]: p = 64u + 32*is_im + c_hi',
    f = v*256 + r''  (c' = 32u + v + 64*c_hi').  top->is_im=0, bot->is_im=1."""
    out = np.empty((128, 32, RPC), F32)
    xt = top[rs].reshape(RPC, 32, 2, 32)      # r'', c_hi', u, v
    xb = bot[rs].reshape(RPC, 32, 2, 32)
    for u in range(2):
        out[64 * u:64 * u + 32] = xt[:, :, u, :].transpose(1, 2, 0)
        out[64 * u + 32:64 * u + 64] = xb[:, :, u, :].transpose(1, 2, 0)
    return np.ascontiguousarray(out.reshape(128, 32 * RPC)).astype(BF16)


def _h_layout(top, bot):
    """[4096 m, 512 k_c] -> [128, 32768]: p = m_hi + 64*is,
    f = chunk*8192 + k_cc*64 + m_lo."""
    out = np.empty((128, NCHUNK, KCHUNK, 64), np.float64)
    t = top.reshape(64, 64, NCHUNK, KCHUNK)   # m_hi, m_lo, chunk, k_cc
    b = bot.reshape(64, 64, NCHUNK, KCHUNK)
    out[:64] = t.transpose(0, 2, 3, 1)
    out[64:] = b.transpose(0, 2, 3, 1)
    return np.ascontiguousarray(out.reshape(128, NCHUNK * KCHUNK * 64)).astype(BF16)


_CONST_CACHE = {}


def _shared_consts():
    if "c" not in _CONST_CACHE:
        cph = const_phase_band()
        hre, him = h_spec_planes()
        WA = stage_a_fwd_mats()                    # [64, 64, 128]
        WA2 = np.concatenate([WA, WA], axis=1)     # [64, 128, 128] both halves
        ID = np.zeros((128, 64), F32)
        ID[:64] = np.eye(64, dtype=F32)
        ID[64:] = np.eye(64, dtype=F32)
        _CONST_CACHE["c"] = dict(
            cosA=np.cos(cph), sinA=np.sin(cph), hre=hre, him=him,
            wa=np.ascontiguousarray(WA2).astype(BF16),
            wb=stage_b_fwd_mat().astype(BF16),
            waiab=stage_a_inv_ab().astype(BF16),
            wai=np.ascontiguousarray(stage_a_inv_mats(1.0 / 4096.0)).astype(BF16),
            wbi=stage_b_inv_mat().astype(BF16),
            id64=ID.astype(BF16),
        )
    return _CONST_CACHE["c"]


def build_core_inputs(optim_param):
    C = _shared_consts()
    bias = bias_band(optim_param).astype(np.float64)
    in_maps = []
    for c in range(NC):
        rs = slice(c * RPC, (c + 1) * RPC)
        ks = slice(c * CPC, (c + 1) * CPC)
        in_maps.append({
            "wa": C["wa"], "wb": C["wb"], "waiab": C["waiab"], "wai": C["wai"],
            "wbi": C["wbi"], "id64": C["id64"],
            "k1": _syn_layout_pair(C["cosA"].astype(F32), C["sinA"].astype(F32), rs),
            "k2": _syn_layout_pair((-C["sinA"]).astype(F32), C["cosA"].astype(F32), rs),
            "bt": _syn_layout_pair(bias.astype(F32), bias.astype(F32), rs),
            "ha": _h_layout(C["hre"][:, ks], C["him"][:, ks]),
            "hb": _h_layout(C["him"][:, ks], C["hre"][:, ks]),
        })
    return in_maps


# ============================================================================
#                              bass / tile program
# ============================================================================
_NC_CACHE = {}


def _build_nc():
    import concourse.bacc as bacc
    import concourse.mybir as mybir
    import concourse.tile as tile

    dtf = mybir.dt.float32
    dtb = mybir.dt.bfloat16
    nc = bacc.Bacc("TRN2", target_bir_lowering=False, debug=False,
                   num_devices=NC)

    D = {}
    def din(name, shape, dt=dtb):
        D[name] = nc.dram_tensor(name, list(shape), dt, kind="ExternalInput").ap()
    din("wa", (64, 128, 128)); din("wb", (128, 128))
    din("waiab", (128, 64 * 2 * 128)); din("wai", (64, 128, 128))
    din("wbi", (128, 64)); din("id64", (128, 64))
    din("k1", (128, 32 * RPC)); din("k2", (128, 32 * RPC)); din("bt", (128, 32 * RPC))
    din("ha", (128, CPC * 64)); din("hb", (128, CPC * 64))
    D["a1i_re"] = nc.dram_tensor("a1i_re", [V, CPC], dtb).ap()
    D["a1i_im"] = nc.dram_tensor("a1i_im", [V, CPC], dtb).ap()
    D["a1o_re"] = nc.dram_tensor("a1o_re", [V, CPC], dtb).ap()
    D["a1o_im"] = nc.dram_tensor("a1o_im", [V, CPC], dtb).ap()
    D["a2i_re"] = nc.dram_tensor("a2i_re", [N, RPC3], dtb).ap()
    D["a2i_im"] = nc.dram_tensor("a2i_im", [N, RPC3], dtb).ap()
    D["a2o_re"] = nc.dram_tensor("a2o_re", [N, RPC3], dtb).ap()
    D["a2o_im"] = nc.dram_tensor("a2o_im", [N, RPC3], dtb).ap()
    D["out"] = nc.dram_tensor("out", [RPC3, BAND_W], dtf, kind="ExternalOutput").ap()

    with tile.TileContext(nc) as tc:
        _emit(nc, tc, mybir, D)
    nc.compile()
    return nc


def _emit(nc, tc, mybir, D):
    dtf = mybir.dt.float32
    dtb = mybir.dt.bfloat16
    AF = mybir.ActivationFunctionType
    HALF_PI = float(np.pi / 2)
    tgl = [0]

    def drain(dst, src):
        tgl[0] ^= 1
        (nc.vector.tensor_copy if tgl[0] else nc.scalar.copy)(dst, src)

    with (
        tc.tile_pool(name="c1", bufs=1) as c1,
        tc.tile_pool(name="big", bufs=3) as bpool,
        tc.tile_pool(name="ps", bufs=5, space="PSUM") as ppool,
        tc.tile_pool(name="pst", bufs=3, space="PSUM") as ptpool,
    ):
        # ---- constants resident all phases ----
        wa_t = c1.tile([128, 64 * 128], dtb, tag="wa")
        nc.sync.dma_start(wa_t[:].rearrange("k (m n) -> k m n", n=128),
                          D["wa"].rearrange("m k n -> k m n"))
        wb_t = c1.tile([128, 128], dtb, tag="wb")
        nc.sync.dma_start(wb_t[:], D["wb"])
        id_t = c1.tile([128, 64], dtb, tag="id")
        nc.sync.dma_start(id_t[:], D["id64"])
        halfpi = c1.tile([128, 1], dtf, tag="hp")
        nc.vector.memset(halfpi[:], HALF_PI)

        def WA(j, base):
            return wa_t[base:base + 64, j * 128:(j + 1) * 128]

        # ---------- digit transpose + scatter-drain helper ----------
        def transp(t_src, t_dst, npen):
            """t_src [128, 64*npen] (p = dlo+64is, f = pen*64 + j) ->
            t_dst [128, 64*npen] (p = j + 64is, f = pen*64 + dlo)."""
            dv = t_dst[:].rearrange("p (pr pen q) -> p pr pen q", pen=2, q=64)
            npair = npen // 2
            for g in range(0, npair, 4):
                kk = min(4, npair - g)
                # separate banks: concurrent PE writes from different row
                # groups into one PSUM bank fault HW
                psa = ptpool.tile([128, 256], dtb, tag="pst")
                psb = ptpool.tile([128, 256], dtb, tag="pst")
                for k in range(kk):
                    pp = g + k
                    src = t_src[:, pp * 128:(pp + 1) * 128]
                    nc.tensor.transpose(psa[:, k * 64:(k + 1) * 64],
                                        src[0:64], id_t[0:64])
                    nc.tensor.transpose(psb[:, k * 64:(k + 1) * 64],
                                        src[64:128], id_t[64:128])
                psav = psa[:].rearrange("p (k q) -> p k q", k=4)
                psbv = psb[:].rearrange("p (k q) -> p k q", k=4)
                for pen in range(2):
                    drain(dv[0:64, g:g + kk, pen, :],
                          psav[64 * pen:64 * pen + 64, 0:kk, :])
                    drain(dv[64:128, g:g + kk, pen, :],
                          psbv[64 * pen:64 * pen + 64, 0:kk, :])

        def stage_a_fwd(t1, Fv, npen):
            """u-interleaved stage A: alternating row-halves across 2 banks so
            LDWEIGHTS of one half overlaps MATMULs of the other."""
            t1w = t1[:].rearrange("p (pen j) -> p pen j", j=64)
            for g in range(8):
                psA = ppool.tile([128, 4 * npen], dtf, tag="ps")
                psB = ppool.tile([128, 4 * npen], dtf, tag="ps")
                for q in range(4):
                    v = 4 * g + q
                    nc.tensor.matmul(psA[:, q * npen:(q + 1) * npen],
                                     WA(v, 0), Fv[0:64, v, :],
                                     start=True, stop=True)
                    nc.tensor.matmul(psB[:, q * npen:(q + 1) * npen],
                                     WA(32 + v, 64), Fv[64:128, v, :],
                                     start=True, stop=True)
                # j-inner iteration: contiguous dst runs, strided PSUM reads
                drain(t1w[:, :, 4 * g:4 * g + 4],
                      psA[:].rearrange("p (q pen) -> p pen q", q=4))
                drain(t1w[:, :, 32 + 4 * g:32 + 4 * g + 4],
                      psB[:].rearrange("p (q pen) -> p pen q", q=4))

        # ============================ P1 ============================
        NPEN1 = 128
        for half in range(2):
            F = bpool.tile([128, 32 * NPEN1], dtb, tag="big")
            with tc.tile_pool(name="syn", bufs=2) as spool:
                for sub in range(2):     # 64-pencil synthesis slices
                    NS = 64
                    fsl = slice(sub * 32 * NS, (sub + 1) * 32 * NS)
                    rsl = slice(half * NPEN1 + sub * NS,
                                half * NPEN1 + (sub + 1) * NS)
                    def dsl(d):
                        return d.rearrange("p (v r) -> p v r", r=RPC)[:, :, rsl]
                    k1t = spool.tile([128, 32 * NS], dtb, tag="k1")
                    k2t = spool.tile([128, 32 * NS], dtb, tag="k2")
                    btt = spool.tile([128, 32 * NS], dtb, tag="bt")
                    sbt = spool.tile([128, 32 * NS], dtb, tag="sb")
                    nc.scalar.dma_start(k1t[:], dsl(D["k1"]))
                    nc.scalar.dma_start(k2t[:], dsl(D["k2"]))
                    nc.scalar.dma_start(btt[:], dsl(D["bt"]))
                    nc.scalar.activation(sbt[:], btt[:], AF.Sin)
                    nc.scalar.activation(btt[:], btt[:], AF.Sin, bias=halfpi[:])
                    nc.vector.tensor_mul(btt[:], k1t[:], btt[:])
                    nc.vector.tensor_mul(sbt[:], k2t[:], sbt[:])
                    nc.vector.tensor_add(
                        F[:].rearrange("p (v r) -> p v r", r=NPEN1)
                        [:, :, sub * 64:(sub + 1) * 64], btt[:], sbt[:])
            # F: p = 64u+32is+c_hi', f = v*128 + pen   (pen local 128)
            Fv = F[:].rearrange("p (v r) -> p v r", r=NPEN1)
            t1 = bpool.tile([128, 64 * NPEN1], dtb, tag="big")
            stage_a_fwd(t1, Fv, NPEN1)
            t2 = bpool.tile([128, 64 * NPEN1], dtb, tag="big")
            transp(t1, t2, NPEN1)
            t3 = bpool.tile([128, 64 * NPEN1], dtb, tag="big")
            for m in range((64 * NPEN1) // 512):
                ps = ppool.tile([128, 512], dtf, tag="ps")
                nc.tensor.matmul(ps[:], wb_t[:], t2[:, m * 512:(m + 1) * 512],
                                 start=True, stop=True)
                drain(t3[:, m * 512:(m + 1) * 512], ps[:])
            # out: t3 [p=k_hi+64is, f=pen*64+k_lo] -> a1i [(s*256+r'')*512 + khl*64+k_lo]
            for isim, nm in ((0, "a1i_re"), (1, "a1i_im")):
                dvw = D[nm].rearrange("(s r) (khl kl) -> s khl r kl", s=8, khl=8)
                for s in range(8):
                    nc.sync.dma_start(
                        dvw[s, :, half * NPEN1:(half + 1) * NPEN1, :],
                        t3[64 * isim + 8 * s:64 * isim + 8 * s + 8, :]
                        .rearrange("p (pen kl) -> p pen kl", kl=64))

        # ============================ A2A 1 ============================
        rg = [list(range(NC))]
        nc.gpsimd.collective_compute("AllToAll", mybir.AluOpType.bypass,
                                     ins=[D["a1i_re"]], outs=[D["a1o_re"]],
                                     replica_groups=rg)
        nc.gpsimd.collective_compute("AllToAll", mybir.AluOpType.bypass,
                                     ins=[D["a1i_im"]], outs=[D["a1o_im"]],
                                     replica_groups=rg)

        # ============================ P2 ============================
        with (
            tc.tile_pool(name="c2", bufs=1) as c2,
            tc.tile_pool(name="lp", bufs=1) as lpool,
            tc.tile_pool(name="hp", bufs=2) as hpool,
            tc.tile_pool(name="kp", bufs=1) as kpool,
            tc.tile_pool(name="sp2", bufs=3) as sp2,
        ):
            waiab_t = c2.tile([128, 64 * 2 * 128], dtb, tag="waiab")
            nc.sync.dma_start(waiab_t[:], D["waiab"])
            wbi_t = c2.tile([128, 64], dtb, tag="wbi")
            nc.sync.dma_start(wbi_t[:], D["wbi"])

            def WAIA(j):
                return waiab_t[:, (2 * j) * 128:(2 * j + 1) * 128]

            def WAIB(j):
                return waiab_t[:, (2 * j + 1) * 128:(2 * j + 2) * 128]

            for chunk in range(NCHUNK):
                L = lpool.tile([128, 32 * KCHUNK], dtb, tag="L")
                for isim, nm in ((0, "a1o_re"), (1, "a1o_im")):
                    av = D[nm].rearrange("(rh u v) k -> u rh v k", u=2, v=32)
                    for u in range(2):
                        eng = nc.sync if u == 0 else nc.scalar
                        eng.dma_start(
                            L[64 * u + 32 * isim:64 * u + 32 * isim + 32, :]
                            .rearrange("p (v kc) -> p v kc", kc=KCHUNK),
                            av[u, :, :, chunk * KCHUNK:(chunk + 1) * KCHUNK])
                # col-FFT stage A (mats by r_lo = 32u+v)
                t1 = bpool.tile([128, 64 * KCHUNK], dtb, tag="big")
                Lv = L[:].rearrange("p (v kc) -> p v kc", kc=KCHUNK)
                stage_a_fwd(t1, Lv, KCHUNK)
                t2 = bpool.tile([128, 64 * KCHUNK], dtb, tag="big")
                transp(t1, t2, KCHUNK)
                # col-FFT stage B + H-mult -> mA/mB [p, f = kc_cc*64 + m_lo]
                mA = kpool.tile([128, 64 * KCHUNK], dtb, tag="mA")
                mB = kpool.tile([128, 64 * KCHUNK], dtb, tag="mB")
                for hh in range(2):
                    hsA = hpool.tile([128, 4096], dtb, tag="hA")
                    hsB = hpool.tile([128, 4096], dtb, tag="hB")
                    off = chunk * 64 * KCHUNK + hh * 4096
                    nc.scalar.dma_start(hsA[:], D["ha"][:, off:off + 4096])
                    nc.scalar.dma_start(hsB[:], D["hb"][:, off:off + 4096])
                    for mm in range(8):
                        sl = slice((hh * 8 + mm) * 512, (hh * 8 + mm + 1) * 512)
                        hsl = slice(mm * 512, (mm + 1) * 512)
                        ps = ppool.tile([128, 512], dtf, tag="ps")
                        nc.tensor.matmul(ps[:], wb_t[:], t2[:, sl],
                                         start=True, stop=True)
                        sbs = sp2.tile([128, 512], dtb, tag="sbs")
                        nc.scalar.copy(sbs[:], ps[:])
                        nc.vector.tensor_mul(mA[:, sl], sbs[:], hsA[:, hsl])
                        nc.vector.tensor_mul(mB[:, sl], sbs[:], hsB[:, hsl])
                # col-IFFT stage A: per m_lo digit, 2 accumulated matmuls
                # (WAIA @ mA + WAIB @ mB) absorb the H recombine.
                mAv = mA[:].rearrange("p (kc ml) -> p ml kc", ml=64)
                mBv = mB[:].rearrange("p (kc ml) -> p ml kc", ml=64)
                ti = bpool.tile([128, 64 * KCHUNK], dtb, tag="big")
                for g in range(16):
                    ps = ppool.tile([128, 512], dtf, tag="ps")
                    for q in range(4):
                        j = 4 * g + q
                        nc.tensor.matmul(ps[:, q * KCHUNK:(q + 1) * KCHUNK],
                                         WAIA(j), mAv[:, j, :],
                                         start=True, stop=False)
                        nc.tensor.matmul(ps[:, q * KCHUNK:(q + 1) * KCHUNK],
                                         WAIB(j), mBv[:, j, :],
                                         start=False, stop=True)
                    drain(ti[:].rearrange("p (pen j) -> p pen j", j=64)[:, :, 4 * g:4 * g + 4],
                          ps[:].rearrange("p (q pen) -> p pen q", q=4))
                tj = bpool.tile([128, 64 * KCHUNK], dtb, tag="big")
                transp(ti, tj, KCHUNK)
                # col-IFFT stage B (pruned out 24 rows)
                tk = bpool.tile([64, 64 * KCHUNK], dtb, tag="big")
                for m in range((64 * KCHUNK) // 512):
                    ps = ppool.tile([128, 512], dtf, tag="ps")
                    nc.tensor.matmul(ps[0:64, :], wbi_t[:],
                                     tj[:, m * 512:(m + 1) * 512],
                                     start=True, stop=True)
                    drain(tk[0:12, m * 512:(m + 1) * 512], ps[0:12, :])
                    drain(tk[32:44, m * 512:(m + 1) * 512], ps[32:44, :])
                # out: tk [p=idx+32is, f=k_cc*64+r_lo] -> a2i [(s*512+kc)*96 + idx*8+rl3]
                for isim, nm in ((0, "a2i_re"), (1, "a2i_im")):
                    dvw = D[nm].rearrange("(s kc) (idx rl3) -> s idx kc rl3",
                                          s=8, idx=12)
                    tv = tk[32 * isim:32 * isim + 12, :].rearrange(
                        "p (kc s rl3) -> p kc s rl3", s=8, rl3=8)
                    for s in range(8):
                        nc.gpsimd.dma_start(
                            dvw[s, :, chunk * KCHUNK:(chunk + 1) * KCHUNK, :],
                            tv[:, :, s, :])

        # ============================ A2A 2 ============================
        nc.gpsimd.collective_compute("AllToAll", mybir.AluOpType.bypass,
                                     ins=[D["a2i_re"]], outs=[D["a2o_re"]],
                                     replica_groups=rg)
        nc.gpsimd.collective_compute("AllToAll", mybir.AluOpType.bypass,
                                     ins=[D["a2i_im"]], outs=[D["a2o_im"]],
                                     replica_groups=rg)

        # ============================ P3 ============================
        with (
            tc.tile_pool(name="c2b", bufs=1) as c2b,
            tc.tile_pool(name="lp3", bufs=1) as lp3,
            tc.tile_pool(name="to3", bufs=1) as to3,
        ):
            wai_t = c2b.tile([128, 64 * 128], dtb, tag="wai3")
            nc.sync.dma_start(wai_t[:].rearrange("k (m n) -> k m n", n=128),
                              D["wai"].rearrange("m k n -> k m n"))
            wbi_t = c2b.tile([128, 64], dtb, tag="wbi3")
            nc.sync.dma_start(wbi_t[:], D["wbi"])
            LB = lp3.tile([128, 64 * RPC3], dtb, tag="LB")
            for isim, nm in ((0, "a2o_re"), (1, "a2o_im")):
                nc.sync.dma_start(
                    LB[64 * isim:64 * isim + 64, :]
                    .rearrange("p (kl r) -> p kl r", r=RPC3),
                    D[nm].rearrange("(kh kl) r -> kh kl r", kl=64))
            # stage A: contract k_hi, mats by k_lo, out digit c_lo; N=96
            t1 = bpool.tile([128, 64 * RPC3], dtb, tag="big")
            for g in range(13):
                ps = ppool.tile([128, 512], dtf, tag="ps")
                qn = min(5, 64 - 5 * g)
                for q in range(qn):
                    j = 5 * g + q
                    nc.tensor.matmul(ps[:, q * RPC3:(q + 1) * RPC3],
                                     wai_t[:, j * 128:(j + 1) * 128],
                                     LB[:, j * RPC3:(j + 1) * RPC3],
                                     start=True, stop=True)
                drain(t1[:].rearrange("p (pen j) -> p pen j", j=64)[:, :, 5 * g:5 * g + qn],
                      ps[:, 0:qn * RPC3].rearrange("p (q pen) -> p pen q", pen=RPC3))
            t2 = bpool.tile([128, 64 * RPC3], dtb, tag="big")
            transp(t1, t2, RPC3)
            # stage B pruned + |.|^2
            tout = to3.tile([12, 64 * RPC3], dtf, tag="bigout")
            for m in range((64 * RPC3) // 512):
                ps = ppool.tile([128, 512], dtf, tag="ps")
                nc.tensor.matmul(ps[0:64, :], wbi_t[:],
                                 t2[:, m * 512:(m + 1) * 512],
                                 start=True, stop=True)
                sq1 = to3.tile([12, 512], dtf, tag="sq1")
                sq2 = to3.tile([12, 512], dtf, tag="sq2")
                nc.scalar.activation(sq1[:], ps[0:12, :], AF.Square)
                nc.scalar.activation(sq2[:], ps[32:44, :], AF.Square)
                nc.vector.tensor_add(tout[:, m * 512:(m + 1) * 512],
                                     sq1[:], sq2[:])
            nc.sync.dma_start(
                D["out"].rearrange("r (ci cl) -> ci r cl", ci=12),
                tout[:].rearrange("p (r cl) -> p r cl", cl=64))


# ============================================================================
#                                   kernel()
# ============================================================================
def kernel(optim_param, _trace=False):
    from concourse.bass_utils import run_bass_kernel_spmd
    if "nc" not in _NC_CACHE:
        _NC_CACHE["nc"] = _build_nc()
    nc = _NC_CACHE["nc"]
    in_maps = build_core_inputs(np.asarray(optim_param, F32))
    res = run_bass_kernel_spmd(nc, in_maps, list(range(NC)), trace=_trace)
    outs = [res.results[c]["out"] for c in range(NC)]      # [96, 768] each
    band = np.empty((BAND_W, BAND_W), np.float64)
    for c in range(NC):
        o = np.asarray(outs[c], np.float64)                # rows idx*8+rl3
        for idx in range(NSEL):
            band[64 * idx + 8 * c:64 * idx + 8 * c + 8, :] = o[8 * idx:8 * idx + 8, :]
    region = band[CROP_OFF:CROP_OFF + WCROP, CROP_OFF:CROP_OFF + WCROP]
    out = (region / region.sum()).astype(F32)[None, None]
    if _trace:
        return out, res
    return out


# revision 26
# speedup vs baseline: 1.0481x; 1.0481x over previous
"""Trainium2 Bass kernel for nn_BaseCamera_1589137899573.

Computes PSF of a phase-mask camera:
  field = aperture * exp(i*(const_phase + spline_bias))   (4096^2, nonzero on central 2048^2)
  psf   = |IFFT2( FFT2(field) * Hs )|^2                   (Hs = ifftshift(exp(i*H_phase)))
  out   = crop 728x728, normalize by sum.

Distribution over 8 NeuronCores (bf16 compute, fp32 PSUM accumulation):
  P1: band rows (2048) split 256/core; synthesize field rows + row-FFT (radix-64
      two-stage matmul DFT, twiddles folded into per-digit stage-A matrices).
  A2A: AllToAll -> each core holds 512 spectral columns x 2048 rows.
  P2: col-FFT + H-multiply + col-IFFT per 128-column chunk. The complex H
      multiply is 2 full-width DVE muls (tables [hre;him], [him;hre]); the
      re/im recombine is absorbed into doubled inverse stage-A weights that
      accumulate in PSUM.
  A2A2 + P3: row-IFFT for 96 of the 768 band rows per core, |.|^2.
  Host: assemble, crop to 728^2, normalize.
"""

import numpy as np
from ml_dtypes import bfloat16

# ---------------- problem constants (hardcoded; must match reference) -------
N = 4096              # WAVE_RES
V = 2048              # VALID_RES (band size)
B0 = 1024             # band start (pad)
PITCH = 2e-6
SENSOR_D = N * PITCH
D1 = 0.05
D2 = 0.05
FOCAL = D1 * D2 / (D1 + D2)
WCROP = 728
LAM = 5.32e-7
UP = 2
TWO_PI = 2.0 * np.pi
K_WAVE = TWO_PI / LAM

CROP_S = N // 2 - WCROP // 2 + 1          # 1685
RHI_LO, RHI_HI = CROP_S // 64, (CROP_S + WCROP - 1) // 64   # 26, 37
NSEL = RHI_HI - RHI_LO + 1                # 12 selected high-digit values
BAND_LO = 64 * RHI_LO                     # 1664
BAND_W = 64 * NSEL                        # 768
CROP_OFF = CROP_S - BAND_LO               # 21

NC = 8                # cores
RPC = V // NC         # 256 band rows per core in P1
CPC = N // NC         # 512 spectral cols per core in P2
KCHUNK = 128          # P2 k_c chunk
NCHUNK = CPC // KCHUNK  # 4
RPC3 = BAND_W // NC   # 96 rows per core in P3

F32 = np.float32
BF16 = bfloat16


# ---------------- small host helpers ----------------------------------------
def _thomas(r):
    """diag=4 off-diag=1 tridiagonal solve, float32 to mirror reference."""
    n = r.shape[0]
    cp = np.zeros(n, np.float32)
    dp = np.zeros(n, np.float32)
    c_prev = np.float32(0.0)
    d_prev = np.float32(0.0)
    for i in range(n):
        den = np.float32(4.0) - c_prev
        c_prev = np.float32(1.0) / den
        d_prev = (r[i] - d_prev) / den
        cp[i] = c_prev
        dp[i] = d_prev
    x = np.zeros(n, np.float32)
    x_next = np.float32(0.0)
    for i in range(n - 1, -1, -1):
        x_next = dp[i] - cp[i] * x_next
        x[i] = x_next
    return x


def spline_quadrant(optim_param):
    """q[i,j] = natural-cubic-spline(mp_log) at r=sqrt((i+.5)^2+(j+.5)^2), [1024,1024]."""
    p = np.asarray(optim_param, np.float32)
    mp = np.repeat(p, UP)
    y = np.concatenate([mp, np.zeros(V // 2, np.float32)])       # len 2048
    n = y.shape[0]
    rhs = (6.0 * (y[2:].astype(np.float64) - 2.0 * y[1:-1] + y[:-2])).astype(np.float32)
    M = np.concatenate([np.zeros(1, np.float32), _thomas(rhs), np.zeros(1, np.float32)])
    half = V // 2
    coord = np.arange(half, dtype=np.float32) + 0.5
    r = np.sqrt(coord[:, None] ** 2 + coord[None, :] ** 2)
    ind = np.clip(np.floor(r).astype(np.int64), 0, n - 2)
    t = r - ind.astype(np.float32)
    y0, y1 = y[ind], y[ind + 1]
    m0, m1 = M[ind], M[ind + 1]
    b = (y1 - y0) - (2.0 * m0 + m1) / 6.0
    return y0 + t * (b + t * (m0 / 2.0 + t * (m1 - m0) / 6.0))


def bias_band(optim_param):
    """Full mirrored bias map on the 2048^2 band."""
    q = spline_quadrant(optim_param)
    row = np.concatenate([q[:, ::-1], q], axis=1)
    return np.concatenate([row[::-1, :], row], axis=0)          # [2048, 2048]


def const_phase_band():
    """(input_phase + lens_phase) mod 2pi on the 2048^2 band."""
    coords = (PITCH * (np.arange(N, dtype=np.float32) - N // 2)).astype(np.float32)
    cb = coords[B0:B0 + V].astype(np.float64)
    r2 = cb[:, None] ** 2 + cb[None, :] ** 2
    ph = np.float64(K_WAVE) * r2 * (1.0 / (2 * D1) - 1.0 / (2 * FOCAL))
    return np.mod(ph, TWO_PI).astype(np.float64)


def h_spec_planes():
    """ifftshifted transfer function exp(i*H_phase): returns (re, im) [4096,4096] f64."""
    fx = ((np.arange(1, N + 1, dtype=np.float32) - np.float32(N / 2)) / np.float32(SENSOR_D)).astype(np.float32)
    FY, FX = np.meshgrid(fx, fx, indexing="ij")
    arg = np.maximum((np.float32(1.0 / LAM)) ** 2 - FX.astype(np.float64) ** 2 - FY.astype(np.float64) ** 2, 0.0)
    w1 = np.sqrt(arg).astype(np.float32)
    hp = (np.float32(TWO_PI) * w1 * np.float32(D2)).astype(np.float32).astype(np.float64)
    hre = np.cos(hp)
    him = np.sin(hp)
    hre = np.fft.ifftshift(hre)
    him = np.fft.ifftshift(him)
    return hre, him


# ---------------- DFT stage matrices (complex->real 2x blocks) ---------------
def _c2r_lhsT(E):
    """Complex matrix E [out m, in k] -> real lhsT [2k, 2m] for out=lhsT.T@rhs.

    Input layout: partitions [re(k) | im(k)], output partitions [re(m) | im(m)].
    """
    m, k = E.shape
    W = np.zeros((2 * k, 2 * m), np.float64)
    W[:k, :m] = E.real.T
    W[k:, :m] = -E.imag.T
    W[:k, m:] = E.imag.T
    W[k:, m:] = E.real.T
    return W.astype(F32)


def stage_a_fwd_mats():
    """WA[c_lo]: [64, 128] real; contracts c_hi' (32 band-high-digits), out k_lo."""
    klo = np.arange(64)[:, None]
    chi = np.arange(32)[None, :]
    mats = []
    for c_lo in range(64):
        E = np.exp(-2j * np.pi * ((16 + chi) * klo % 64) / 64.0) \
            * np.exp(-2j * np.pi * (c_lo * klo) / 4096.0)
        mats.append(_c2r_lhsT(E))
    return np.stack(mats)                                        # [64, 64, 128]


def stage_b_fwd_mat():
    """WB: [128, 128]; contracts c_lo (64), out k_hi. Pure DFT-64."""
    khi = np.arange(64)[:, None]
    clo = np.arange(64)[None, :]
    E = np.exp(-2j * np.pi * (clo * khi % 64) / 64.0)
    return _c2r_lhsT(E)                                          # [128, 128]


def stage_a_inv_mats(scale):
    """WAI[m_lo]: [128, 128]; contracts m_hi (full 64), out r_lo, +sign, *scale."""
    rlo = np.arange(64)[:, None]
    mhi = np.arange(64)[None, :]
    mats = []
    for m_lo in range(64):
        E = np.exp(2j * np.pi * (mhi * rlo % 64) / 64.0) \
            * np.exp(2j * np.pi * (m_lo * rlo) / 4096.0) * scale
        mats.append(_c2r_lhsT(E))
    return np.stack(mats)                                        # [64, 128, 128]


def stage_a_inv_ab():
    """Doubled inverse stage-A weights absorbing the H-multiply recombine.

    mA[p] = (p<64 ? re*hre : im*him), mB[p] = (p<64 ? re*him : im*hre).
    SH_re = mA_lo - mA_hi ; SH_im = mB_lo + mB_hi.
    T_j = WAIA_j.T @ mA + WAIB_j.T @ mB  (PSUM accumulated), where
    WAIA_j = [WAI_j[0:64]; -WAI_j[0:64]], WAIB_j = [WAI_j[64:128]; WAI_j[64:128]].
    Returns [128, 64*2*128] f32 (cast later).
    """
    WAI = stage_a_inv_mats(1.0 / 4096.0)                         # [64, 128, 128]
    out = np.empty((128, 64, 2, 128), np.float32)
    for j in range(64):
        out[0:64, j, 0, :] = WAI[j][0:64]
        out[64:128, j, 0, :] = -WAI[j][0:64]
        out[0:64, j, 1, :] = WAI[j][64:128]
        out[64:128, j, 1, :] = WAI[j][64:128]
    return np.ascontiguousarray(out.reshape(128, 64 * 2 * 128))


def stage_b_inv_mat():
    """WBI: [128, 64]; contracts m_lo, out r_hi in {26..37}.
    Output partitions: re at 0:12, im at 32:44 (32-aligned for engine APs)."""
    rhi = np.arange(RHI_LO, RHI_HI + 1)[:, None]
    mlo = np.arange(64)[None, :]
    E = np.exp(2j * np.pi * (mlo * rhi % 64) / 64.0)
    W = _c2r_lhsT(E)                                             # [128, 24]
    out = np.zeros((128, 64), F32)
    out[:, 0:12] = W[:, 0:12]
    out[:, 32:44] = W[:, 12:24]
    return out


# ============================================================================
#                        host-side per-core input builders
# ============================================================================
def _syn_layout_pair(top, bot, rs):
    """[256, 2048] slabs -> [128, 8192]: p = 64u + 32*is_im + c_hi',
    f = v*256 + r''  (c' = 32u + v + 64*c_hi').  top->is_im=0, bot->is_im=1."""
    out = np.empty((128, 32, RPC), F32)
    xt = top[rs].reshape(RPC, 32, 2, 32)      # r'', c_hi', u, v
    xb = bot[rs].reshape(RPC, 32, 2, 32)
    for u in range(2):
        out[64 * u:64 * u + 32] = xt[:, :, u, :].transpose(1, 2, 0)
        out[64 * u + 32:64 * u + 64] = xb[:, :, u, :].transpose(1, 2, 0)
    return np.ascontiguousarray(out.reshape(128, 32 * RPC)).astype(BF16)


def _h_layout(top, bot):
    """[4096 m, 512 k_c] -> [128, 32768]: p = m_hi + 64*is,
    f = chunk*8192 + k_cc*64 + m_lo."""
    out = np.empty((128, NCHUNK, KCHUNK, 64), np.float64)
    t = top.reshape(64, 64, NCHUNK, KCHUNK)   # m_hi, m_lo, chunk, k_cc
    b = bot.reshape(64, 64, NCHUNK, KCHUNK)
    out[:64] = t.transpose(0, 2, 3, 1)
    out[64:] = b.transpose(0, 2, 3, 1)
    return np.ascontiguousarray(out.reshape(128, NCHUNK * KCHUNK * 64)).astype(BF16)


_CONST_CACHE = {}


def _shared_consts():
    if "c" not in _CONST_CACHE:
        cph = const_phase_band()
        hre, him = h_spec_planes()
        WA = stage_a_fwd_mats()                    # [64, 64, 128]
        WA2 = np.concatenate([WA, WA], axis=1)     # [64, 128, 128] both halves
        ID = np.zeros((128, 64), F32)
        ID[:64] = np.eye(64, dtype=F32)
        ID[64:] = np.eye(64, dtype=F32)
        _CONST_CACHE["c"] = dict(
            cosA=np.cos(cph), sinA=np.sin(cph), hre=hre, him=him,
            wa=np.ascontiguousarray(WA2).astype(BF16),
            wb=stage_b_fwd_mat().astype(BF16),
            waiab=stage_a_inv_ab().astype(BF16),
            wai=np.ascontiguousarray(stage_a_inv_mats(1.0 / 4096.0)).astype(BF16),
            wbi=stage_b_inv_mat().astype(BF16),
            id64=ID.astype(BF16),
        )
    return _CONST_CACHE["c"]


def build_core_inputs(optim_param):
    C = _shared_consts()
    bias = bias_band(optim_param).astype(np.float64)
    in_maps = []
    for c in range(NC):
        rs = slice(c * RPC, (c + 1) * RPC)
        ks = slice(c * CPC, (c + 1) * CPC)
        in_maps.append({
            "wa": C["wa"], "wb": C["wb"], "waiab": C["waiab"], "wai": C["wai"],
            "wbi": C["wbi"], "id64": C["id64"],
            "k1": _syn_layout_pair(C["cosA"].astype(F32), C["sinA"].astype(F32), rs),
            "k2": _syn_layout_pair((-C["sinA"]).astype(F32), C["cosA"].astype(F32), rs),
            "bt": _syn_layout_pair(bias.astype(F32), bias.astype(F32), rs),
            "ha": _h_layout(C["hre"][:, ks], C["him"][:, ks]),
            "hb": _h_layout(C["him"][:, ks], C["hre"][:, ks]),
        })
    return in_maps


# ============================================================================
#                              bass / tile program
# ============================================================================
_NC_CACHE = {}


def _build_nc():
    import concourse.bacc as bacc
    import concourse.mybir as mybir
    import concourse.tile as tile

    dtf = mybir.dt.float32
    dtb = mybir.dt.bfloat16
    nc = bacc.Bacc("TRN2", target_bir_lowering=False, debug=False,
                   num_devices=NC)

    D = {}
    def din(name, shape, dt=dtb):
        D[name] = nc.dram_tensor(name, list(shape), dt, kind="ExternalInput").ap()
    din("wa", (64, 128, 128)); din("wb", (128, 128))
    din("waiab", (128, 64 * 2 * 128)); din("wai", (64, 128, 128))
    din("wbi", (128, 64)); din("id64", (128, 64))
    din("k1", (128, 32 * RPC)); din("k2", (128, 32 * RPC)); din("bt", (128, 32 * RPC))
    din("ha", (128, CPC * 64)); din("hb", (128, CPC * 64))
    D["a1i"] = nc.dram_tensor("a1i", [V, 2 * CPC], dtb).ap()
    D["a1o"] = nc.dram_tensor("a1o", [V, 2 * CPC], dtb).ap()
    D["a2i"] = nc.dram_tensor("a2i", [N, 2 * RPC3], dtb).ap()
    D["a2o"] = nc.dram_tensor("a2o", [N, 2 * RPC3], dtb).ap()
    D["out"] = nc.dram_tensor("out", [RPC3, BAND_W], dtf, kind="ExternalOutput").ap()

    with tile.TileContext(nc) as tc:
        _emit(nc, tc, mybir, D)
    nc.compile()
    return nc


def _emit(nc, tc, mybir, D):
    dtf = mybir.dt.float32
    dtb = mybir.dt.bfloat16
    AF = mybir.ActivationFunctionType
    HALF_PI = float(np.pi / 2)
    tgl = [0]

    def drain(dst, src):
        tgl[0] ^= 1
        (nc.vector.tensor_copy if tgl[0] else nc.scalar.copy)(dst, src)

    with (
        tc.tile_pool(name="c1", bufs=1) as c1,
        tc.tile_pool(name="big", bufs=3) as bpool,
        tc.tile_pool(name="ps", bufs=5, space="PSUM") as ppool,
        tc.tile_pool(name="pst", bufs=3, space="PSUM") as ptpool,
    ):
        # ---- constants resident all phases ----
        wa_t = c1.tile([128, 64 * 128], dtb, tag="wa")
        nc.sync.dma_start(wa_t[:].rearrange("k (m n) -> k m n", n=128),
                          D["wa"].rearrange("m k n -> k m n"))
        wb_t = c1.tile([128, 128], dtb, tag="wb")
        nc.sync.dma_start(wb_t[:], D["wb"])
        id_t = c1.tile([128, 64], dtb, tag="id")
        nc.sync.dma_start(id_t[:], D["id64"])
        halfpi = c1.tile([128, 1], dtf, tag="hp")
        nc.vector.memset(halfpi[:], HALF_PI)

        def WA(j, base):
            return wa_t[base:base + 64, j * 128:(j + 1) * 128]

        # ---------- digit transpose + scatter-drain helper ----------
        def transp(t_src, t_dst, npen):
            """t_src [128, 64*npen] (p = dlo+64is, f = pen*64 + j) ->
            t_dst [128, 64*npen] (p = j + 64is, f = pen*64 + dlo)."""
            dv = t_dst[:].rearrange("p (pr pen q) -> p pr pen q", pen=2, q=64)
            npair = npen // 2
            for g in range(0, npair, 4):
                kk = min(4, npair - g)
                # separate banks: concurrent PE writes from different row
                # groups into one PSUM bank fault HW
                psa = ptpool.tile([128, 256], dtb, tag="pst")
                psb = ptpool.tile([128, 256], dtb, tag="pst")
                for k in range(kk):
                    pp = g + k
                    src = t_src[:, pp * 128:(pp + 1) * 128]
                    nc.tensor.transpose(psa[:, k * 64:(k + 1) * 64],
                                        src[0:64], id_t[0:64])
                    nc.tensor.transpose(psb[:, k * 64:(k + 1) * 64],
                                        src[64:128], id_t[64:128])
                psav = psa[:].rearrange("p (k q) -> p k q", k=4)
                psbv = psb[:].rearrange("p (k q) -> p k q", k=4)
                for pen in range(2):
                    drain(dv[0:64, g:g + kk, pen, :],
                          psav[64 * pen:64 * pen + 64, 0:kk, :])
                    drain(dv[64:128, g:g + kk, pen, :],
                          psbv[64 * pen:64 * pen + 64, 0:kk, :])

        def stage_a_fwd(t1, Fv, npen):
            """u-interleaved stage A: alternating row-halves across 2 banks so
            LDWEIGHTS of one half overlaps MATMULs of the other."""
            t1w = t1[:].rearrange("p (pen j) -> p pen j", j=64)
            for g in range(8):
                psA = ppool.tile([128, 4 * npen], dtf, tag="ps")
                psB = ppool.tile([128, 4 * npen], dtf, tag="ps")
                for q in range(4):
                    v = 4 * g + q
                    nc.tensor.matmul(psA[:, q * npen:(q + 1) * npen],
                                     WA(v, 0), Fv[0:64, v, :],
                                     start=True, stop=True)
                    nc.tensor.matmul(psB[:, q * npen:(q + 1) * npen],
                                     WA(32 + v, 64), Fv[64:128, v, :],
                                     start=True, stop=True)
                # j-inner iteration: contiguous dst runs, strided PSUM reads
                drain(t1w[:, :, 4 * g:4 * g + 4],
                      psA[:].rearrange("p (q pen) -> p pen q", q=4))
                drain(t1w[:, :, 32 + 4 * g:32 + 4 * g + 4],
                      psB[:].rearrange("p (q pen) -> p pen q", q=4))

        # ============================ P1 ============================
        NPEN1 = 128
        for half in range(2):
            F = bpool.tile([128, 32 * NPEN1], dtb, tag="big")
            with tc.tile_pool(name="syn", bufs=2) as spool:
                for sub in range(2):     # 64-pencil synthesis slices
                    NS = 64
                    fsl = slice(sub * 32 * NS, (sub + 1) * 32 * NS)
                    rsl = slice(half * NPEN1 + sub * NS,
                                half * NPEN1 + (sub + 1) * NS)
                    def dsl(d):
                        return d.rearrange("p (v r) -> p v r", r=RPC)[:, :, rsl]
                    k1t = spool.tile([128, 32 * NS], dtb, tag="k1")
                    k2t = spool.tile([128, 32 * NS], dtb, tag="k2")
                    btt = spool.tile([128, 32 * NS], dtb, tag="bt")
                    sbt = spool.tile([128, 32 * NS], dtb, tag="sb")
                    nc.scalar.dma_start(k1t[:], dsl(D["k1"]))
                    nc.scalar.dma_start(k2t[:], dsl(D["k2"]))
                    nc.scalar.dma_start(btt[:], dsl(D["bt"]))
                    nc.scalar.activation(sbt[:], btt[:], AF.Sin)
                    nc.scalar.activation(btt[:], btt[:], AF.Sin, bias=halfpi[:])
                    nc.vector.tensor_mul(btt[:], k1t[:], btt[:])
                    nc.vector.tensor_mul(sbt[:], k2t[:], sbt[:])
                    nc.vector.tensor_add(
                        F[:].rearrange("p (v r) -> p v r", r=NPEN1)
                        [:, :, sub * 64:(sub + 1) * 64], btt[:], sbt[:])
            # F: p = 64u+32is+c_hi', f = v*128 + pen   (pen local 128)
            Fv = F[:].rearrange("p (v r) -> p v r", r=NPEN1)
            t1 = bpool.tile([128, 64 * NPEN1], dtb, tag="big")
            stage_a_fwd(t1, Fv, NPEN1)
            t2 = bpool.tile([128, 64 * NPEN1], dtb, tag="big")
            transp(t1, t2, NPEN1)
            t3 = bpool.tile([128, 64 * NPEN1], dtb, tag="big")
            for m in range((64 * NPEN1) // 512):
                ps = ppool.tile([128, 512], dtf, tag="ps")
                nc.tensor.matmul(ps[:], wb_t[:], t2[:, m * 512:(m + 1) * 512],
                                 start=True, stop=True)
                drain(t3[:, m * 512:(m + 1) * 512], ps[:])
            # out: t3 [p=k_hi+64is, f=pen*64+k_lo] -> a1i [(s*256+r'')*512 + khl*64+k_lo]
            dvw = D["a1i"].rearrange("(s r) (i khl kl) -> s i khl r kl",
                                     s=8, i=2, khl=8)
            for isim in (0, 1):
                for s in range(8):
                    nc.sync.dma_start(
                        dvw[s, isim, :, half * NPEN1:(half + 1) * NPEN1, :],
                        t3[64 * isim + 8 * s:64 * isim + 8 * s + 8, :]
                        .rearrange("p (pen kl) -> p pen kl", kl=64))

        # ============================ A2A 1 ============================
        rg = [list(range(NC))]
        nc.gpsimd.collective_compute("AllToAll", mybir.AluOpType.bypass,
                                     ins=[D["a1i"]], outs=[D["a1o"]],
                                     replica_groups=rg)

        # ============================ P2 ============================
        with (
            tc.tile_pool(name="c2", bufs=1) as c2,
            tc.tile_pool(name="lp", bufs=1) as lpool,
            tc.tile_pool(name="hp", bufs=2) as hpool,
            tc.tile_pool(name="kp", bufs=1) as kpool,
            tc.tile_pool(name="sp2", bufs=3) as sp2,
        ):
            waiab_t = c2.tile([128, 64 * 2 * 128], dtb, tag="waiab")
            nc.sync.dma_start(waiab_t[:], D["waiab"])
            wbi_t = c2.tile([128, 64], dtb, tag="wbi")
            nc.sync.dma_start(wbi_t[:], D["wbi"])

            def WAIA(j):
                return waiab_t[:, (2 * j) * 128:(2 * j + 1) * 128]

            def WAIB(j):
                return waiab_t[:, (2 * j + 1) * 128:(2 * j + 2) * 128]

            for chunk in range(NCHUNK):
                L = lpool.tile([128, 32 * KCHUNK], dtb, tag="L")
                av = D["a1o"].rearrange("(rh u v) (i k) -> i u rh v k",
                                        u=2, v=32, i=2)
                for isim in (0, 1):
                    for u in range(2):
                        nc.sync.dma_start(
                            L[64 * u + 32 * isim:64 * u + 32 * isim + 32, :]
                            .rearrange("p (v kc) -> p v kc", kc=KCHUNK),
                            av[isim, u, :, :, chunk * KCHUNK:(chunk + 1) * KCHUNK])
                # col-FFT stage A (mats by r_lo = 32u+v)
                t1 = bpool.tile([128, 64 * KCHUNK], dtb, tag="big")
                Lv = L[:].rearrange("p (v kc) -> p v kc", kc=KCHUNK)
                stage_a_fwd(t1, Lv, KCHUNK)
                t2 = bpool.tile([128, 64 * KCHUNK], dtb, tag="big")
                transp(t1, t2, KCHUNK)
                # col-FFT stage B + H-mult -> mA/mB [p, f = kc_cc*64 + m_lo]
                mA = kpool.tile([128, 64 * KCHUNK], dtb, tag="mA")
                mB = kpool.tile([128, 64 * KCHUNK], dtb, tag="mB")
                for hh in range(2):
                    hsA = hpool.tile([128, 4096], dtb, tag="hA")
                    hsB = hpool.tile([128, 4096], dtb, tag="hB")
                    off = chunk * 64 * KCHUNK + hh * 4096
                    nc.scalar.dma_start(hsA[:], D["ha"][:, off:off + 4096])
                    nc.scalar.dma_start(hsB[:], D["hb"][:, off:off + 4096])
                    for mm in range(8):
                        sl = slice((hh * 8 + mm) * 512, (hh * 8 + mm + 1) * 512)
                        hsl = slice(mm * 512, (mm + 1) * 512)
                        ps = ppool.tile([128, 512], dtf, tag="ps")
                        nc.tensor.matmul(ps[:], wb_t[:], t2[:, sl],
                                         start=True, stop=True)
                        sbs = sp2.tile([128, 512], dtb, tag="sbs")
                        nc.scalar.copy(sbs[:], ps[:])
                        nc.vector.tensor_mul(mA[:, sl], sbs[:], hsA[:, hsl])
                        nc.vector.tensor_mul(mB[:, sl], sbs[:], hsB[:, hsl])
                # col-IFFT stage A: per m_lo digit, 2 accumulated matmuls
                # (WAIA @ mA + WAIB @ mB) absorb the H recombine.
                mAv = mA[:].rearrange("p (kc ml) -> p ml kc", ml=64)
                mBv = mB[:].rearrange("p (kc ml) -> p ml kc", ml=64)
                ti = bpool.tile([128, 64 * KCHUNK], dtb, tag="big")
                for g in range(16):
                    ps = ppool.tile([128, 512], dtf, tag="ps")
                    for q in range(4):
                        j = 4 * g + q
                        nc.tensor.matmul(ps[:, q * KCHUNK:(q + 1) * KCHUNK],
                                         WAIA(j), mAv[:, j, :],
                                         start=True, stop=False)
                        nc.tensor.matmul(ps[:, q * KCHUNK:(q + 1) * KCHUNK],
                                         WAIB(j), mBv[:, j, :],
                                         start=False, stop=True)
                    drain(ti[:].rearrange("p (pen j) -> p pen j", j=64)[:, :, 4 * g:4 * g + 4],
                          ps[:].rearrange("p (q pen) -> p pen q", q=4))
                tj = bpool.tile([128, 64 * KCHUNK], dtb, tag="big")
                transp(ti, tj, KCHUNK)
                # col-IFFT stage B (pruned out 24 rows)
                tk = bpool.tile([64, 64 * KCHUNK], dtb, tag="big")
                for m in range((64 * KCHUNK) // 512):
                    ps = ppool.tile([128, 512], dtf, tag="ps")
                    nc.tensor.matmul(ps[0:64, :], wbi_t[:],
                                     tj[:, m * 512:(m + 1) * 512],
                                     start=True, stop=True)
                    drain(tk[0:12, m * 512:(m + 1) * 512], ps[0:12, :])
                    drain(tk[32:44, m * 512:(m + 1) * 512], ps[32:44, :])
                # out: tk [p=idx+32is, f=k_cc*64+r_lo] -> a2i [(s*512+kc)*96 + idx*8+rl3]
                dvw2 = D["a2i"].rearrange("(s kc) (i idx rl3) -> s i idx kc rl3",
                                          s=8, i=2, idx=12)
                for isim in (0, 1):
                    tv = tk[32 * isim:32 * isim + 12, :].rearrange(
                        "p (kc s rl3) -> p kc s rl3", s=8, rl3=8)
                    for s in range(8):
                        nc.sync.dma_start(
                            dvw2[s, isim, :, chunk * KCHUNK:(chunk + 1) * KCHUNK, :],
                            tv[:, :, s, :])

        # ============================ A2A 2 ============================
        nc.gpsimd.collective_compute("AllToAll", mybir.AluOpType.bypass,
                                     ins=[D["a2i"]], outs=[D["a2o"]],
                                     replica_groups=rg)

        # ============================ P3 ============================
        with (
            tc.tile_pool(name="c2b", bufs=1) as c2b,
            tc.tile_pool(name="lp3", bufs=1) as lp3,
            tc.tile_pool(name="to3", bufs=1) as to3,
        ):
            wai_t = c2b.tile([128, 64 * 128], dtb, tag="wai3")
            nc.sync.dma_start(wai_t[:].rearrange("k (m n) -> k m n", n=128),
                              D["wai"].rearrange("m k n -> k m n"))
            wbi_t = c2b.tile([128, 64], dtb, tag="wbi3")
            nc.sync.dma_start(wbi_t[:], D["wbi"])
            LB = lp3.tile([128, 64 * RPC3], dtb, tag="LB")
            av3 = D["a2o"].rearrange("(kh kl) (i r) -> i kh kl r", kl=64, i=2)
            for isim in (0, 1):
                nc.sync.dma_start(
                    LB[64 * isim:64 * isim + 64, :]
                    .rearrange("p (kl r) -> p kl r", r=RPC3),
                    av3[isim])
            # stage A: contract k_hi, mats by k_lo, out digit c_lo; N=96
            t1 = bpool.tile([128, 64 * RPC3], dtb, tag="big")
            for g in range(13):
                ps = ppool.tile([128, 512], dtf, tag="ps")
                qn = min(5, 64 - 5 * g)
                for q in range(qn):
                    j = 5 * g + q
                    nc.tensor.matmul(ps[:, q * RPC3:(q + 1) * RPC3],
                                     wai_t[:, j * 128:(j + 1) * 128],
                                     LB[:, j * RPC3:(j + 1) * RPC3],
                                     start=True, stop=True)
                drain(t1[:].rearrange("p (pen j) -> p pen j", j=64)[:, :, 5 * g:5 * g + qn],
                      ps[:, 0:qn * RPC3].rearrange("p (q pen) -> p pen q", pen=RPC3))
            t2 = bpool.tile([128, 64 * RPC3], dtb, tag="big")
            transp(t1, t2, RPC3)
            # stage B pruned + |.|^2
            tout = to3.tile([12, 64 * RPC3], dtf, tag="bigout")
            for m in range((64 * RPC3) // 512):
                ps = ppool.tile([128, 512], dtf, tag="ps")
                nc.tensor.matmul(ps[0:64, :], wbi_t[:],
                                 t2[:, m * 512:(m + 1) * 512],
                                 start=True, stop=True)
                sq1 = to3.tile([12, 512], dtf, tag="sq1")
                sq2 = to3.tile([12, 512], dtf, tag="sq2")
                nc.scalar.activation(sq1[:], ps[0:12, :], AF.Square)
                nc.scalar.activation(sq2[:], ps[32:44, :], AF.Square)
                nc.vector.tensor_add(tout[:, m * 512:(m + 1) * 512],
                                     sq1[:], sq2[:])
            nc.sync.dma_start(
                D["out"].rearrange("r (ci cl) -> ci r cl", ci=12),
                tout[:].rearrange("p (r cl) -> p r cl", cl=64))


# ============================================================================
#                                   kernel()
# ============================================================================
def kernel(optim_param, _trace=False):
    from concourse.bass_utils import run_bass_kernel_spmd
    if "nc" not in _NC_CACHE:
        _NC_CACHE["nc"] = _build_nc()
    nc = _NC_CACHE["nc"]
    in_maps = build_core_inputs(np.asarray(optim_param, F32))
    res = run_bass_kernel_spmd(nc, in_maps, list(range(NC)), trace=_trace)
    outs = [res.results[c]["out"] for c in range(NC)]      # [96, 768] each
    band = np.empty((BAND_W, BAND_W), np.float64)
    for c in range(NC):
        o = np.asarray(outs[c], np.float64)                # rows idx*8+rl3
        for idx in range(NSEL):
            band[64 * idx + 8 * c:64 * idx + 8 * c + 8, :] = o[8 * idx:8 * idx + 8, :]
    region = band[CROP_OFF:CROP_OFF + WCROP, CROP_OFF:CROP_OFF + WCROP]
    out = (region / region.sum()).astype(F32)[None, None]
    if _trace:
        return out, res
    return out
